# revision 55
# baseline (speedup 1.0000x reference)
"""Trainium2 Bass kernel for nn_BaselineOut (article/option additive-attention MRC head).

Contract: kernel(**inputs) takes FULL unsharded inputs (numpy), returns FULL
[32, 5] float32 logits.  Internally: data-parallel over batch across 8 cores
(4 batch items per core), all params replicated.

Math notes (vs reference):
  - oqc gather is done as a one-hot matmul on device.
  - V-projection is pulled out of the attention sum by linearity:
        sum_l softmax_l * (V @ Vw^T + Vb) = (sum_l softmax_l * V) @ Vw^T + Vb
  - Consecutive linear maps with no nonlinearity between are constant-folded
    on host (weight-weight products):
      * aq -> Qp_d: one matmul with Wqv = d_Qw @ a_Vw^T and a folded bias.
      * feats -> logits: per-option folded weights Ff_o = d_Vw^T @ f_w[:,o]^T.
  - softmax logit bias (vb) is dropped: softmax is shift-invariant.
  - exp is computed without max-subtraction: |logit| <= ||vw||_1 ~ 36, well
    inside fp32 exp range.

Precision (validated vs reference on CPU, rel err ~7e-3 vs 2e-2 budget):
  - article K-projection (the 17 GFLOP/core matmul) runs in fp8-e4m3 with
    DoubleRow perf mode (2 fp8 weights per PE cell, 2x matmul throughput —
    measured 213ns per [256x128x512] MM, the fp8 roofline).
  - article softmax scores, tanh output, V-sum operands, and the two bias
    projections (a_Qw, Wqv) are fp8 too: their noise is averaged away by the
    2048-wide article softmax.  The options branch stays bf16 (fp8 there
    fails: softmax over only 32 positions doesn't average the noise away).
  - accumulation is fp32 in PSUM everywhere; softmax sums/normalization fp32.

Structure/scheduling (what made it fast):
  - Article softmax logits are produced as PSUM *columns* (lhsT=tanh tile,
    rhs=score vector), so exp is a cheap [128,8] activation and the weighted
    V-sum becomes PE matmuls against a second, natural-layout fp8 article
    copy (u += An[lc]^T @ sc[lc]), with the softmax sums riding along as a
    ones-column matmul.  No DVE reduction anywhere in the article loop.
  - PSUM accumulation-group starts reset has_written flags *bank-wide* on
    TRN2: groups sharing a bank must be closed before another group's start
    (this kernel always runs groups back-to-back per bank).
  - The options branch is per-batch-item separable; each item's chain
    (normalize u -> biasO -> bias-add -> tanh) plus its K-projection is
    interleaved into the article loop right after that item's article tiles,
    leaving only the last item + scores/V-sum/final-linear in the tail.
  - Option softmax sums/normalization use 32-row block-mask matmuls; the
    option V-sum uses block-diagonal masked score columns on the PE.
  - Three DMA queues (sync=weights/options, gpsimd=article streams), large
    2KB-line transfers, and late emission of tail-only weights keep the
    ~330GB/s shared DMA fabric ahead of the PE.
  - biasA matmuls are interleaved into tile 0's co-loop so the first article
    tile doesn't wait for the full a_Qw load.
"""

import functools
import sys

import numpy as np

sys.path.insert(0, "/opt/trn_rl_repo")

import ml_dtypes  # noqa: E402

import concourse.bass as bass  # noqa: E402
from concourse import bacc  # noqa: E402
import concourse.tile as tile  # noqa: E402
from concourse import mybir  # noqa: E402
from concourse.bass import ds, ts  # noqa: E402

B, LA, LQ, LO, H, OUT = 32, 2048, 64, 32, 1024, 5
NCORES = 8
BL = B // NCORES  # 4 batch items per core
NOPT = 5
F32 = mybir.dt.float32
BF16 = mybir.dt.bfloat16
FP8 = mybir.dt.float8e4
LH = 1024  # article l-tile (half of one batch item's article)
HT = LA // LH  # 2 halves per batch item
C = H // 128  # 8 h-chunks
CP = C // 2  # 4 ci pairs for DoubleRow
BO = BL * NOPT  # 20 (b, option) pairs per core
OL = BO * LO  # 640 flattened option columns
AF = mybir.ActivationFunctionType
ALU = mybir.AluOpType
AX = mybir.AxisListType
OUTP = 8  # final-linear out dim padded
DR = mybir.MatmulPerfMode.DoubleRow
# per-batch-item options work per article tile (starts after wdk/OT land)
OPT_SCHED = {3: [0], 4: [1], 5: [2]}


def build_nc() -> bass.Bass:
    nc = bacc.Bacc("TRN2", target_bir_lowering=False, debug=False)

    # ---- DRAM I/O (per-core shard; names are the in_map keys) ----
    art8 = nc.dram_tensor("art8", [BL, H, LA], FP8, kind="ExternalInput").ap()
    art16 = nc.dram_tensor("art16", [BL, LA, H], FP8, kind="ExternalInput").ap()
    optT = nc.dram_tensor("optT", [BL, H, NOPT, LO], BF16, kind="ExternalInput").ap()
    optN = nc.dram_tensor("optN", [BL * NOPT * LO, H], BF16, kind="ExternalInput").ap()
    qcd = nc.dram_tensor("qc", [BL, LQ, H], BF16, kind="ExternalInput").ap()
    ohd = nc.dram_tensor("oh", [LQ, BL], BF16, kind="ExternalInput").ap()
    wQa = nc.dram_tensor("aQwT", [H, H], FP8, kind="ExternalInput").ap()
    wKa = nc.dram_tensor("aKwT", [H, H], FP8, kind="ExternalInput").ap()
    wQV = nc.dram_tensor("qvwT", [H, H], FP8, kind="ExternalInput").ap()
    wKd = nc.dram_tensor("dKwT", [H, H], BF16, kind="ExternalInput").ap()
    vwad = nc.dram_tensor("vwaT", [128, C], FP8, kind="ExternalInput").ap()
    vwdd = nc.dram_tensor("vwdT", [128, C], BF16, kind="ExternalInput").ap()
    qkbd = nc.dram_tensor("qkbT", [128, C], F32, kind="ExternalInput").ap()
    qvbd = nc.dram_tensor("qvbT", [128, C], F32, kind="ExternalInput").ap()
    fwd = nc.dram_tensor("fwT", [128, NOPT, C, OUTP], BF16, kind="ExternalInput").ap()
    fbd = nc.dram_tensor("fb", [BL, OUTP], F32, kind="ExternalInput").ap()
    m32d = nc.dram_tensor("m32", [128, 4], BF16, kind="ExternalInput").ap()
    m32Td = nc.dram_tensor("m32T", [4, 128], BF16, kind="ExternalInput").ap()
    outd = nc.dram_tensor("out", [BL, OUT], F32, kind="ExternalOutput").ap()

    with (
        tile.TileContext(nc) as tc,
        nc.allow_low_precision(reason="fp8/bf16 operands; PE accumulates fp32"),
    ):
        with (
            tc.tile_pool(name="one", bufs=1) as one,
            tc.tile_pool(name="stream", bufs=3) as stream,
            tc.tile_pool(name="nstream", bufs=3) as nstream,
            tc.tile_pool(name="mtp", bufs=2) as mtp,
            tc.tile_pool(name="scp", bufs=2) as scp,
            tc.tile_pool(name="scrp", bufs=1) as scrp,
            tc.tile_pool(name="pacc", bufs=2, space="PSUM") as pacc,
            tc.tile_pool(name="plgc", bufs=1, space="PSUM") as plgc,
            tc.tile_pool(name="pub", bufs=1, space="PSUM") as pub,
            tc.tile_pool(name="pkpd", bufs=1, space="PSUM") as pkpd,
            tc.tile_pool(name="psml", bufs=1, space="PSUM") as psml,
        ):
            # ---------- sync (HWDGE) queue: qc, weights, smalls, then An ------
            oht = one.tile([LQ, BL], BF16, tag="oht")
            nc.sync.dma_start(out=oht, in_=ohd)
            qct = one.tile([LQ, BL, H], BF16, tag="qct")
            nc.sync.dma_start(out=qct, in_=qcd.rearrange("b l h -> l b h"))
            # a_Qw: first column-chunk alone (so biasA(0) can start early),
            # the rest as one large-line transfer
            wq = one.tile([128, C, H], FP8, tag="wq")
            nc.sync.dma_start(
                out=wq[:, :, ts(0, 128)],
                in_=wQa[:, ts(0, 128)].rearrange("(c p) o -> p c o", p=128),
            )
            nc.sync.dma_start(
                out=wq[:, :, ds(128, H - 128)],
                in_=wQa[:, ds(128, H - 128)].rearrange("(c p) o -> p c o", p=128),
            )
            vwa = one.tile([128, C], FP8, tag="vwa")
            nc.sync.dma_start(out=vwa, in_=vwad)
            vwd = one.tile([128, C], BF16, tag="vwd")
            nc.sync.dma_start(out=vwd, in_=vwdd)
            qkb = one.tile([128, C], F32, tag="qkb")
            nc.sync.dma_start(out=qkb, in_=qkbd)
            qvb = one.tile([128, C], F32, tag="qvb")
            nc.sync.dma_start(out=qvb, in_=qvbd)
            fw = one.tile([128, NOPT, C, OUTP], BF16, tag="fw")
            fb = one.tile([BL, OUTP], F32, tag="fb")
            # options-phase weights load later (emitted at tiles 2/3 so the
            # early bandwidth goes to the article streams)
            wdk = one.tile([128, C, H], BF16, tag="wdk")
            OT = one.tile([128, C, BL, NOPT, LO], BF16, tag="OT")
            wqv = one.tile([128, C, H], FP8, tag="wqv")
            On = one.tile([128, NOPT, H], BF16, tag="On")

            # ---------- gpsimd (SWDGE) queue: article streams (fp8 + bf16) ----
            wk8 = one.tile([128, C, H], FP8, tag="wk8")
            nc.gpsimd.dma_start(
                out=wk8[:, :, ts(0, 128)],
                in_=wKa[:, ts(0, 128)].rearrange("(c p) o -> p c o", p=128),
            )

            # ---------- persistent accumulators / small tensors ----------
            oqcT = one.tile([128, C, BL], FP8, tag="oqcT")
            biasA = one.tile([128, C, BL], F32, tag="biasA")
            biasO = one.tile([128, C, BL], F32, tag="biasO")
            uTun = one.tile([128, C, BL], F32, tag="uTun")
            uT = one.tile([128, BL, C], FP8, tag="uT")
            mdt_pre = one.tile([128, C, OL], BF16, tag="mdt_pre")
            ones128 = one.tile([128, 1], FP8, tag="ones128")
            nc.vector.memset(ones128, 1.0)
            onesrow = one.tile([1, 128], BF16, tag="onesrow")
            nc.vector.memset(onesrow, 1.0)
            # 32-row block masks for per-(b,o) option-softmax sums on the PE
            mask32 = one.tile([128, 4], BF16, tag="mask32")
            mask32T = one.tile([4, 128], BF16, tag="mask32T")
            # u accumulator: per-half-tile columns h*C+co (all groups are
            # closed within one tile: HW start_tensor_calc resets has_written
            # flags bank-wide, so open groups must never interleave with
            # other groups' starts in the same bank); softmax partial sums in
            # columns 2C+h*NLC.. (partition 0)
            pu = pub.tile([128, 2 * C + 2 * (LH // 128)], F32, tag="pu")
            ssums = one.tile([1, BL], F32, tag="ssums")

            # ---------- gather oqc via one-hot matmul ----------
            for c in range(C):
                po = psml.tile([128, BL], F32, tag="sml")
                for b in range(BL):
                    nc.tensor.matmul(
                        po[:, b : b + 1],
                        lhsT=qct[:, b, ts(c, 128)],
                        rhs=oht[:, b : b + 1],
                        start=True,
                        stop=True,
                    )
                nc.vector.tensor_copy(oqcT[:, c, :], po)

            # ---------- article branch ----------
            def bias_a_block(co):
                # biasA[:, co, :] = a_Qw @ oqc^T + (a_Qb + a_Kb), chunk co
                pq = psml.tile([128, BL], F32, tag="sml")
                for ci in range(C):
                    nc.tensor.matmul(
                        pq,
                        lhsT=wq[:, ci, ts(co, 128)],
                        rhs=oqcT[:, ci, :],
                        start=(ci == 0),
                        stop=(ci == C - 1),
                    )
                nc.vector.tensor_scalar_add(biasA[:, co, :], pq, qkb[:, co : co + 1])

            QB = OL // 4  # 160 columns = one batch item

            def kpd_b(j):
                # options K-projection for batch item j -> mdt_pre[:, :, j]
                OTf = OT.rearrange("p c b o l -> p c (b o l)")
                for co in range(C):
                    pool, ktag = (pkpd, "kpdq") if co % 2 == 0 else (psml, "sml")
                    kpd = pool.tile([128, QB], F32, tag=ktag)
                    for ci in range(C):
                        nc.tensor.matmul(
                            kpd,
                            lhsT=wdk[:, ci, ts(co, 128)],
                            rhs=OTf[:, ci, ds(j * QB, QB)],
                            start=(ci == 0),
                            stop=(ci == C - 1),
                        )
                    nc.vector.tensor_copy(mdt_pre[:, co, ds(j * QB, QB)], kpd)

            mdt = scrp.tile([128, C, OL], BF16, tag="scr")

            def opt_chain(j):
                # per-batch-item options pipeline: normalize u, biasO,
                # bias-add + tanh (runs inside the article loop)
                rsb = one.tile([1, BL], F32, tag="rsb")
                nc.vector.reciprocal(rsb[:, j : j + 1], ssums[:, j : j + 1])
                rsb16 = one.tile([1, BL], BF16, tag="rsb16")
                nc.vector.tensor_copy(rsb16[:, j : j + 1], rsb[:, j : j + 1])
                psb = psml.tile([128, 1], F32, tag="sml")
                nc.tensor.matmul(
                    psb, lhsT=onesrow, rhs=rsb16[:, j : j + 1], start=True, stop=True
                )
                nc.vector.tensor_mul(
                    uT[:, j, :], uTun[:, :, j], psb.broadcast_to((128, C))
                )
                pq2 = psml.tile([128, C], F32, tag="sml")
                for co in range(C):
                    for ci in range(C):
                        nc.tensor.matmul(
                            pq2[:, co : co + 1],
                            lhsT=wqv[:, ci, ts(co, 128)],
                            rhs=uT[:, j, ci : ci + 1],
                            start=(ci == 0),
                            stop=(ci == C - 1),
                        )
                nc.vector.tensor_add(biasO[:, :, j], pq2, qvb)
                sl = ds(j * QB, QB)
                nc.vector.tensor_add(
                    mdt[:, :, sl],
                    mdt_pre[:, :, sl],
                    biasO[:, :, j : j + 1].broadcast_to((128, C, QB)),
                )
                nc.scalar.activation(mdt[:, :, sl], mdt[:, :, sl], AF.Tanh)

            NLC = LH // 128  # 8 l-chunks of 128 per tile
            tile_idx = 0
            for b in range(BL):
                for h in range(HT):
                    if b == 0:
                        # b0 in halves so tile 0 starts ASAP
                        T8b = stream.tile([128, C, LH], FP8, tag="stream")
                        nc.gpsimd.dma_start(
                            out=T8b,
                            in_=art8[0, :, ds(h * LH, LH)].rearrange(
                                "(c p) l -> p c l", p=128
                            ),
                        )
                        T8 = T8b
                    elif h == 0:
                        # whole-b fp8 article in one large-line transfer
                        T8b = stream.tile([128, C, LA], FP8, tag="stream")
                        nc.gpsimd.dma_start(
                            out=T8b,
                            in_=art8[b].rearrange("(c p) l -> p c l", p=128),
                        )
                        T8 = T8b[:, :, ds(0, LH)]
                    else:
                        T8 = T8b[:, :, ds(h * LH, LH)]
                    if tile_idx == 0:
                        # rest of the fp8 K-weights in two transfers: few
                        # issue slots (~1us each on gpsimd) while still
                        # unblocking tile 0's middle co-blocks early
                        nc.gpsimd.dma_start(
                            out=wk8[:, :, ds(128, 384)],
                            in_=wKa[:, ds(128, 384)].rearrange(
                                "(c p) o -> p c o", p=128
                            ),
                        )
                        nc.gpsimd.dma_start(
                            out=wk8[:, :, ds(512, 512)],
                            in_=wKa[:, ds(512, 512)].rearrange(
                                "(c p) o -> p c o", p=128
                            ),
                        )
                    if tile_idx == 2:
                        nc.sync.dma_start(
                            out=wdk, in_=wKd.rearrange("(c p) o -> p c o", p=128)
                        )
                        for bb in range(BL):
                            nc.sync.dma_start(
                                out=OT[:, :, bb],
                                in_=optT[bb].rearrange("(c p) o l -> p c o l", p=128),
                            )
                    if tile_idx == 3:
                        nc.sync.dma_start(
                            out=wqv, in_=wQV.rearrange("(c p) o -> p c o", p=128)
                        )
                        nc.sync.dma_start(
                            out=On, in_=optN.rearrange("(lc p) hh -> p lc hh", p=128)
                        )
                        nc.sync.dma_start(out=fw, in_=fwd)
                        nc.sync.dma_start(out=fb, in_=fbd)
                        nc.sync.dma_start(out=mask32, in_=m32d)
                        nc.sync.dma_start(out=mask32T, in_=m32Td)
                    # natural-layout article for the PE weighted V-sum
                    An = nstream.tile([128, NLC, H], FP8, tag="nstream")
                    nc.gpsimd.dma_start(
                        out=An,
                        in_=art16[b, ds(h * LH, LH), :].rearrange(
                            "(lc p) hh -> p lc hh", p=128
                        ),
                    )
                    mt = mtp.tile([128, C, LH], FP8, tag="mt")
                    for co in range(C):
                        if tile_idx == 0:
                            bias_a_block(co)
                        kp = pacc.tile([128, LH], F32, tag="acc")
                        for lt in range(2):
                            for cp in range(CP):
                                nc.tensor.matmul(
                                    kp[:, ds(lt * 512, 512)],
                                    lhsT=wk8[:, ds(2 * cp, 2), ts(co, 128)],
                                    rhs=T8[:, ds(2 * cp, 2), ds(lt * 512, 512)],
                                    start=(cp == 0),
                                    stop=(cp == CP - 1),
                                    perf_mode=DR,
                                )
                        nc.scalar.activation(
                            mt[:, co], kp, AF.Tanh, bias=biasA[:, co, b : b + 1]
                        )
                    # score logits as PSUM columns; one closed group per lc
                    lgc = plgc.tile([128, NLC], F32, tag="lgc")
                    for lc in range(NLC):
                        for co in range(C):
                            nc.tensor.matmul(
                                lgc[:, lc : lc + 1],
                                lhsT=mt[:, co, ts(lc, 128)],
                                rhs=vwa[:, co : co + 1],
                                start=(co == 0),
                                stop=(co == C - 1),
                            )
                    sc = scp.tile([128, NLC], FP8, tag="sc")
                    nc.scalar.activation(sc, lgc, AF.Exp)
                    # weighted V-sum + softmax-sum on the PE; groups closed
                    # within this tile (columns h*C+co and 2C+h)
                    for co in range(C):
                        for lc in range(NLC):
                            nc.tensor.matmul(
                                pu[:, h * C + co : h * C + co + 1],
                                lhsT=An[:, lc, ts(co, 128)],
                                rhs=sc[:, lc : lc + 1],
                                start=(lc == 0),
                                stop=(lc == NLC - 1),
                            )
                    nc.tensor.matmul(
                        pu[0:1, ds(2 * C + h * NLC, NLC)],
                        lhsT=ones128,
                        rhs=sc,
                        start=True,
                        stop=True,
                    )
                    if h == HT - 1:
                        # finalize this batch item's u and softmax sum now so
                        # its options chain can run inside the loop
                        nc.vector.tensor_copy(uTun[:, :, b], pu[:, 0:C])
                        nc.vector.tensor_add(
                            uTun[:, :, b], uTun[:, :, b], pu[:, C : 2 * C]
                        )
                        nc.vector.tensor_reduce(
                            ssums[:, b : b + 1].unsqueeze(1),
                            pu[0:1, ds(2 * C, 2 * NLC)].unsqueeze(1),
                            axis=AX.X,
                            op=ALU.add,
                        )
                    # per-batch-item options work interleaved into the loop
                    for j in OPT_SCHED.get(tile_idx, []):
                        kpd_b(j)
                        opt_chain(j)
                    if tile_idx == 6:
                        kpd_b(3)
                    tile_idx += 1

            # ---------- options branch tail: last batch item + scores ------
            opt_chain(3)
            # option score logits as PSUM columns (5 ol-chunks of 128),
            # one closed group per chunk
            mdtf = mdt.rearrange("p c ol -> p (c ol)")
            lgdc = plgc.tile([128, NOPT], F32, tag="lgc")
            for lc in range(NOPT):
                for co in range(C):
                    nc.tensor.matmul(
                        lgdc[:, lc : lc + 1],
                        lhsT=mdtf[:, ds(co * OL + lc * 128, 128)],
                        rhs=vwd[:, co : co + 1],
                        start=(co == 0),
                        stop=(co == C - 1),
                    )
            sdc = scp.tile([128, NOPT], BF16, tag="sc")
            nc.scalar.activation(sdc, lgdc, AF.Exp)
            # per-(b,o) softmax sums: 32-row block sums via mask matmuls
            ssml = psml.tile([4, NOPT], F32, tag="sml")
            for lc in range(NOPT):
                nc.tensor.matmul(
                    ssml[:, lc : lc + 1],
                    lhsT=mask32,
                    rhs=sdc[:, lc : lc + 1],
                    start=True,
                    stop=True,
                )
            recs = one.tile([4, NOPT], F32, tag="recs")
            nc.vector.reciprocal(recs, ssml)
            recs16 = one.tile([4, NOPT], BF16, tag="recs16")
            nc.vector.tensor_copy(recs16, recs)
            # replicate 1/sum back to the 32-row blocks, normalize, and mask
            # into block-diagonal score columns
            prep = psml.tile([128, NOPT], F32, tag="sml")
            nc.tensor.matmul(prep, lhsT=mask32T, rhs=recs16, start=True, stop=True)
            sdnc = scp.tile([128, NOPT], BF16, tag="sc")
            nc.vector.tensor_mul(sdnc, sdc, prep)
            bd = one.tile([128, NOPT, 4], BF16, tag="bd")
            nc.vector.tensor_mul(
                bd,
                mask32.unsqueeze(1).broadcast_to((128, NOPT, 4)),
                sdnc.unsqueeze(-1).broadcast_to((128, NOPT, 4)),
            )
            # weighted V-sum on the PE: u_d[h, (b,o)] per (c, lc) block
            udps = pub.tile([128, C * BO], F32, tag="pu")
            for c in range(C):
                for lc in range(NOPT):
                    nc.tensor.matmul(
                        udps[:, ds(c * BO + 4 * lc, 4)],
                        lhsT=On[:, lc, ts(c, 128)],
                        rhs=bd[:, lc, :],
                        start=True,
                        stop=True,
                    )
            u16 = one.tile([128, C, BL, NOPT], BF16, tag="u16")
            nc.vector.tensor_copy(
                u16.rearrange("p c b o -> p c (b o)"),
                udps.rearrange("p (c bo) -> p c bo", c=C),
            )
            pout = psml.tile([BL, OUTP], F32, tag="sml")
            for c in range(C):
                for o in range(NOPT):
                    nc.tensor.matmul(
                        pout,
                        lhsT=u16[:, c, :, o],
                        rhs=fw[:, o, c, :],
                        start=(c == 0 and o == 0),
                        stop=(c == C - 1 and o == NOPT - 1),
                    )
            out_s = one.tile([BL, OUTP], F32, tag="out_s")
            nc.vector.tensor_add(out_s, pout, fb)
            nc.sync.dma_start(out=outd, in_=out_s[:, 0:OUT])

    nc.compile()
    return nc


@functools.lru_cache(maxsize=1)
def get_nc() -> bass.Bass:
    return build_nc()


def make_in_maps(inputs: dict) -> list[dict]:
    bf16 = ml_dtypes.bfloat16
    fp8 = ml_dtypes.float8_e4m3
    art = np.ascontiguousarray(np.asarray(inputs["article_contexts"], np.float32))
    qc = np.asarray(inputs["question_contexts"], np.float32)
    opt = np.ascontiguousarray(np.asarray(inputs["options_embeds"], np.float32))
    idx = np.asarray(inputs["answer_indices"]).astype(np.int64)

    def g(name):
        return np.asarray(inputs[name], np.float32)

    aQwT = np.ascontiguousarray(g("a_Qw").T).astype(fp8)
    aKwT = np.ascontiguousarray(g("a_Kw").T).astype(fp8)
    dKwT = np.ascontiguousarray(g("d_Kw").T).astype(bf16)
    # folded: aq -> options query projection
    Wqv = g("d_Qw") @ g("a_Vw")  # [H, H]
    qvwT = np.ascontiguousarray(Wqv.T).astype(fp8)
    bias_qv = g("d_Qw") @ g("a_Vb") + g("d_Qb") + g("d_Kb")  # [H]
    # folded: per-option final weights
    f_w = g("f_w")  # [OUT, 5H]
    dVwT = g("d_Vw").T
    Ff = np.stack(
        [dVwT @ f_w[:, o * H : (o + 1) * H].T for o in range(NOPT)], axis=0
    )  # [o, H_in, OUT]
    fb_new = g("f_b") + sum(
        f_w[:, o * H : (o + 1) * H] @ g("d_Vb") for o in range(NOPT)
    )  # [OUT]
    fwT = np.zeros((128, NOPT, C, OUTP), np.float32)
    fwT[:, :, :, :OUT] = Ff.reshape(NOPT, C, 128, OUT).transpose(2, 0, 1, 3)

    def colvec(v, dt):  # [H] -> [128, C] chunk-major
        return np.ascontiguousarray(
            np.asarray(v, np.float32).reshape(C, 128).T
        ).astype(dt)

    vwaT = colvec(g("a_vw").reshape(H), fp8)
    vwdT = colvec(g("d_vw").reshape(H), bf16)
    qkbT = colvec(g("a_Qb") + g("a_Kb"), np.float32)
    qvbT = colvec(bias_qv, np.float32)

    artT = np.ascontiguousarray(art.transpose(0, 2, 1)).astype(fp8)  # [B, H, LA]
    artN = art.astype(fp8)  # [B, LA, H] natural layout for the PE V-sum
    optT = np.ascontiguousarray(opt.transpose(0, 3, 1, 2)).astype(bf16)
    onehot = np.zeros((B, LQ), np.float32)
    onehot[np.arange(B), idx] = 1.0

    shared = dict(
        aQwT=aQwT, aKwT=aKwT, qvwT=qvwT, dKwT=dKwT,
        vwaT=vwaT, vwdT=vwdT, qkbT=qkbT, qvbT=qvbT,
        fwT=fwT.astype(bf16),
        fb=np.ascontiguousarray(
            np.tile(np.pad(fb_new.astype(np.float32), (0, 3)).reshape(1, 8), (BL, 1))
        ),
        m32=np.kron(np.eye(4, dtype=np.float32), np.ones((32, 1), np.float32)).astype(
            bf16
        ),
        m32T=np.kron(np.eye(4, dtype=np.float32), np.ones((1, 32), np.float32)).astype(
            bf16
        ),
    )
    in_maps = []
    for r in range(NCORES):
        s = slice(r * BL, (r + 1) * BL)
        m = dict(shared)
        m["art8"] = artT[s]
        m["art16"] = artN[s]
        m["optT"] = optT[s]
        m["optN"] = np.ascontiguousarray(opt[s].reshape(-1, H)).astype(bf16)
        m["qc"] = qc[s].astype(bf16)
        m["oh"] = np.ascontiguousarray(onehot[s].T).astype(bf16)
        in_maps.append(m)
    return in_maps


def run(inputs: dict, trace: bool = False, tmpdir=None):
    from concourse.bass_utils import run_bass_kernel_spmd

    nc = get_nc()
    in_maps = make_in_maps(inputs)
    res = run_bass_kernel_spmd(
        nc, in_maps, core_ids=list(range(NCORES)), trace=trace, tmpdir=tmpdir
    )
    out = np.concatenate([res.results[r]["out"] for r in range(NCORES)], axis=0)
    return out, res


def kernel(**inputs) -> np.ndarray:
    out, _ = run(inputs, trace=False)
    return out


# revision 57
# speedup vs baseline: 1.1862x; 1.1862x over previous
"""Trainium2 Bass kernel for nn_BaselineOut (article/option additive-attention MRC head).

Contract: kernel(**inputs) takes FULL unsharded inputs (numpy), returns FULL
[32, 5] float32 logits.  Internally: data-parallel over batch across 8 cores
(4 batch items per core), all params replicated.

Math notes (vs reference):
  - oqc gather is done as a one-hot matmul on device.
  - V-projection is pulled out of the attention sum by linearity:
        sum_l softmax_l * (V @ Vw^T + Vb) = (sum_l softmax_l * V) @ Vw^T + Vb
  - Consecutive linear maps with no nonlinearity between are constant-folded
    on host (weight-weight products):
      * aq -> Qp_d: one matmul with Wqv = d_Qw @ a_Vw^T and a folded bias.
      * feats -> logits: per-option folded weights Ff_o = d_Vw^T @ f_w[:,o]^T.
  - softmax logit bias (vb) is dropped: softmax is shift-invariant.
  - exp is computed without max-subtraction: |logit| <= ||vw||_1 ~ 36, well
    inside fp32 exp range.

Precision (validated vs reference on CPU, rel err ~7e-3 vs 2e-2 budget):
  - article K-projection (the 17 GFLOP/core matmul) runs in fp8-e4m3 with
    DoubleRow perf mode (2 fp8 weights per PE cell, 2x matmul throughput —
    measured 213ns per [256x128x512] MM, the fp8 roofline).
  - article softmax scores, tanh output, V-sum operands, and the two bias
    projections (a_Qw, Wqv) are fp8 too: their noise is averaged away by the
    2048-wide article softmax.  The options branch stays bf16 (fp8 there
    fails: softmax over only 32 positions doesn't average the noise away).
  - accumulation is fp32 in PSUM everywhere; softmax sums/normalization fp32.

Structure/scheduling (what made it fast):
  - Article softmax logits are produced as PSUM *columns* (lhsT=tanh tile,
    rhs=score vector), so exp is a cheap [128,8] activation and the weighted
    V-sum becomes PE matmuls against a second, natural-layout fp8 article
    copy (u += An[lc]^T @ sc[lc]), with the softmax sums riding along as a
    ones-column matmul.  No DVE reduction anywhere in the article loop.
  - PSUM accumulation-group starts reset has_written flags *bank-wide* on
    TRN2: groups sharing a bank must be closed before another group's start
    (this kernel always runs groups back-to-back per bank).
  - The options branch is per-batch-item separable; each item's chain
    (normalize u -> biasO -> bias-add -> tanh) plus its K-projection is
    interleaved into the article loop right after that item's article tiles,
    leaving only the last item + scores/V-sum/final-linear in the tail.
  - Option softmax sums/normalization use 32-row block-mask matmuls; the
    option V-sum uses block-diagonal masked score columns on the PE.
  - Three DMA queues (sync=weights/options, gpsimd=article streams), large
    2KB-line transfers, and late emission of tail-only weights keep the
    ~330GB/s shared DMA fabric ahead of the PE.
  - biasA matmuls are interleaved into tile 0's co-loop so the first article
    tile doesn't wait for the full a_Qw load.
"""

import functools
import sys

import numpy as np

sys.path.insert(0, "/opt/trn_rl_repo")

import ml_dtypes  # noqa: E402

import concourse.bass as bass  # noqa: E402
from concourse import bacc  # noqa: E402
import concourse.tile as tile  # noqa: E402
from concourse import mybir  # noqa: E402
from concourse.bass import ds, ts  # noqa: E402

B, LA, LQ, LO, H, OUT = 32, 2048, 64, 32, 1024, 5
NCORES = 8
BL = B // NCORES  # 4 batch items per core
NOPT = 5
F32 = mybir.dt.float32
BF16 = mybir.dt.bfloat16
FP8 = mybir.dt.float8e4
LH = 1024  # article l-tile (half of one batch item's article)
HT = LA // LH  # 2 halves per batch item
C = H // 128  # 8 h-chunks
CP = C // 2  # 4 ci pairs for DoubleRow
BO = BL * NOPT  # 20 (b, option) pairs per core
OL = BO * LO  # 640 flattened option columns
AF = mybir.ActivationFunctionType
ALU = mybir.AluOpType
AX = mybir.AxisListType
OUTP = 8  # final-linear out dim padded
DR = mybir.MatmulPerfMode.DoubleRow
# per-batch-item options work per article tile (starts after wdk/OT land)
OPT_SCHED = {3: [0], 4: [1], 5: [2]}


def build_nc() -> bass.Bass:
    nc = bacc.Bacc("TRN2", target_bir_lowering=False, debug=False)

    # ---- DRAM I/O (per-core shard; names are the in_map keys) ----
    art8 = nc.dram_tensor("art8", [BL, H, LA], FP8, kind="ExternalInput").ap()
    art16 = nc.dram_tensor("art16", [BL, LA, H], FP8, kind="ExternalInput").ap()
    optT = nc.dram_tensor("optT", [BL, H, NOPT, LO], BF16, kind="ExternalInput").ap()
    optN = nc.dram_tensor("optN", [BL * NOPT * LO, H], BF16, kind="ExternalInput").ap()
    qcd = nc.dram_tensor("qc", [BL, LQ, H], BF16, kind="ExternalInput").ap()
    ohd = nc.dram_tensor("oh", [LQ, BL], BF16, kind="ExternalInput").ap()
    wQa = nc.dram_tensor("aQwT", [H, H], FP8, kind="ExternalInput").ap()
    wKa = nc.dram_tensor("aKwT", [H, H], FP8, kind="ExternalInput").ap()
    wQV = nc.dram_tensor("qvwT", [H, H], FP8, kind="ExternalInput").ap()
    wKd = nc.dram_tensor("dKwT", [H, H], BF16, kind="ExternalInput").ap()
    vwad = nc.dram_tensor("vwaT", [128, C], FP8, kind="ExternalInput").ap()
    vwdd = nc.dram_tensor("vwdT", [128, C], BF16, kind="ExternalInput").ap()
    qkbd = nc.dram_tensor("qkbT", [128, C], F32, kind="ExternalInput").ap()
    qvbd = nc.dram_tensor("qvbT", [128, C], F32, kind="ExternalInput").ap()
    fwd = nc.dram_tensor("fwT", [128, NOPT, C, OUTP], BF16, kind="ExternalInput").ap()
    fbd = nc.dram_tensor("fb", [BL, OUTP], F32, kind="ExternalInput").ap()
    m32d = nc.dram_tensor("m32", [128, 4], BF16, kind="ExternalInput").ap()
    m32Td = nc.dram_tensor("m32T", [4, 128], BF16, kind="ExternalInput").ap()
    outd = nc.dram_tensor("out", [BL, OUT], F32, kind="ExternalOutput").ap()

    with (
        tile.TileContext(nc) as tc,
        nc.allow_low_precision(reason="fp8/bf16 operands; PE accumulates fp32"),
    ):
        with (
            tc.tile_pool(name="one", bufs=1) as one,
            tc.tile_pool(name="stream", bufs=3) as stream,
            tc.tile_pool(name="nstream", bufs=3) as nstream,
            tc.tile_pool(name="mtp", bufs=2) as mtp,
            tc.tile_pool(name="scp", bufs=2) as scp,
            tc.tile_pool(name="scrp", bufs=1) as scrp,
            tc.tile_pool(name="pacc", bufs=2, space="PSUM") as pacc,
            tc.tile_pool(name="plgc", bufs=1, space="PSUM") as plgc,
            tc.tile_pool(name="pub", bufs=1, space="PSUM") as pub,
            tc.tile_pool(name="pkpd", bufs=1, space="PSUM") as pkpd,
            tc.tile_pool(name="psml", bufs=1, space="PSUM") as psml,
        ):
            # ---------- sync (HWDGE) queue: qc, weights, smalls, then An ------
            oht = one.tile([LQ, BL], BF16, tag="oht")
            nc.sync.dma_start(out=oht, in_=ohd)
            qct = one.tile([LQ, BL, H], BF16, tag="qct")
            nc.sync.dma_start(out=qct, in_=qcd.rearrange("b l h -> l b h"))
            # a_Qw: first column-chunk alone (so biasA(0) can start early),
            # the rest as one large-line transfer
            wq = one.tile([128, C, H], FP8, tag="wq")
            nc.sync.dma_start(
                out=wq[:, :, ts(0, 128)],
                in_=wQa[:, ts(0, 128)].rearrange("(c p) o -> p c o", p=128),
            )
            nc.sync.dma_start(
                out=wq[:, :, ds(128, H - 128)],
                in_=wQa[:, ds(128, H - 128)].rearrange("(c p) o -> p c o", p=128),
            )
            vwa = one.tile([128, C], FP8, tag="vwa")
            nc.sync.dma_start(out=vwa, in_=vwad)
            vwd = one.tile([128, C], BF16, tag="vwd")
            nc.sync.dma_start(out=vwd, in_=vwdd)
            qkb = one.tile([128, C], F32, tag="qkb")
            nc.sync.dma_start(out=qkb, in_=qkbd)
            qvb = one.tile([128, C], F32, tag="qvb")
            nc.sync.dma_start(out=qvb, in_=qvbd)
            fw = one.tile([128, NOPT, C, OUTP], BF16, tag="fw")
            fb = one.tile([BL, OUTP], F32, tag="fb")
            # options-phase weights load later (emitted at tiles 2/3 so the
            # early bandwidth goes to the article streams)
            wdk = one.tile([128, C, H], BF16, tag="wdk")
            OT = one.tile([128, C, BL, NOPT, LO], BF16, tag="OT")
            wqv = one.tile([128, C, H], FP8, tag="wqv")
            On = one.tile([128, NOPT, H], BF16, tag="On")

            # ---------- gpsimd (SWDGE) queue: article streams (fp8 + bf16) ----
            wk8 = one.tile([128, C, H], FP8, tag="wk8")
            nc.gpsimd.dma_start(
                out=wk8[:, :, ts(0, 128)],
                in_=wKa[:, ts(0, 128)].rearrange("(c p) o -> p c o", p=128),
            )

            # ---------- persistent accumulators / small tensors ----------
            oqcT = one.tile([128, C, BL], FP8, tag="oqcT")
            biasA = one.tile([128, C, BL], F32, tag="biasA")
            biasO = one.tile([128, C, BL], F32, tag="biasO")
            uTun = one.tile([128, C, BL], F32, tag="uTun")
            uT = one.tile([128, BL, C], FP8, tag="uT")
            mdt_pre = one.tile([128, C, OL], BF16, tag="mdt_pre")
            ones128 = one.tile([128, 1], FP8, tag="ones128")
            nc.vector.memset(ones128, 1.0)
            onesrow = one.tile([1, 128], BF16, tag="onesrow")
            nc.vector.memset(onesrow, 1.0)
            # 32-row block masks for per-(b,o) option-softmax sums on the PE
            mask32 = one.tile([128, 4], BF16, tag="mask32")
            mask32T = one.tile([4, 128], BF16, tag="mask32T")
            # u accumulator: per-half-tile columns h*C+co (all groups are
            # closed within one tile: HW start_tensor_calc resets has_written
            # flags bank-wide, so open groups must never interleave with
            # other groups' starts in the same bank); softmax partial sums in
            # columns 2C+h*NLC.. (partition 0)
            pu = pub.tile([128, 2 * C + 2 * (LH // 128)], F32, tag="pu")
            ssums = one.tile([1, BL], F32, tag="ssums")

            # ---------- gather oqc via one-hot matmul ----------
            for c in range(C):
                po = psml.tile([128, BL], F32, tag="sml")
                for b in range(BL):
                    nc.tensor.matmul(
                        po[:, b : b + 1],
                        lhsT=qct[:, b, ts(c, 128)],
                        rhs=oht[:, b : b + 1],
                        start=True,
                        stop=True,
                    )
                nc.vector.tensor_copy(oqcT[:, c, :], po)

            # ---------- article branch ----------
            def bias_a_block(co):
                # biasA[:, co, :] = a_Qw @ oqc^T + (a_Qb + a_Kb), chunk co
                pq = psml.tile([128, BL], F32, tag="sml")
                for ci in range(C):
                    nc.tensor.matmul(
                        pq,
                        lhsT=wq[:, ci, ts(co, 128)],
                        rhs=oqcT[:, ci, :],
                        start=(ci == 0),
                        stop=(ci == C - 1),
                    )
                nc.vector.tensor_scalar_add(biasA[:, co, :], pq, qkb[:, co : co + 1])

            QB = OL // 4  # 160 columns = one batch item

            def kpd_b(j):
                # options K-projection for batch item j -> mdt_pre[:, :, j]
                OTf = OT.rearrange("p c b o l -> p c (b o l)")
                for co in range(C):
                    pool, ktag = (pkpd, "kpdq") if co % 2 == 0 else (psml, "sml")
                    kpd = pool.tile([128, QB], F32, tag=ktag)
                    for ci in range(C):
                        nc.tensor.matmul(
                            kpd,
                            lhsT=wdk[:, ci, ts(co, 128)],
                            rhs=OTf[:, ci, ds(j * QB, QB)],
                            start=(ci == 0),
                            stop=(ci == C - 1),
                        )
                    nc.vector.tensor_copy(mdt_pre[:, co, ds(j * QB, QB)], kpd)

            mdt = scrp.tile([128, C, OL], BF16, tag="scr")

            def opt_chain(j):
                # per-batch-item options pipeline: normalize u, biasO,
                # bias-add + tanh (runs inside the article loop)
                rsb = one.tile([1, BL], F32, tag="rsb")
                nc.vector.reciprocal(rsb[:, j : j + 1], ssums[:, j : j + 1])
                rsb16 = one.tile([1, BL], BF16, tag="rsb16")
                nc.vector.tensor_copy(rsb16[:, j : j + 1], rsb[:, j : j + 1])
                psb = psml.tile([128, 1], F32, tag="sml")
                nc.tensor.matmul(
                    psb, lhsT=onesrow, rhs=rsb16[:, j : j + 1], start=True, stop=True
                )
                nc.vector.tensor_mul(
                    uT[:, j, :], uTun[:, :, j], psb.broadcast_to((128, C))
                )
                pq2 = psml.tile([128, C], F32, tag="sml")
                for co in range(C):
                    for ci in range(C):
                        nc.tensor.matmul(
                            pq2[:, co : co + 1],
                            lhsT=wqv[:, ci, ts(co, 128)],
                            rhs=uT[:, j, ci : ci + 1],
                            start=(ci == 0),
                            stop=(ci == C - 1),
                        )
                nc.vector.tensor_add(biasO[:, :, j], pq2, qvb)
                sl = ds(j * QB, QB)
                nc.vector.tensor_add(
                    mdt[:, :, sl],
                    mdt_pre[:, :, sl],
                    biasO[:, :, j : j + 1].broadcast_to((128, C, QB)),
                )
                nc.scalar.activation(mdt[:, :, sl], mdt[:, :, sl], AF.Tanh)

            NLC = LH // 128  # 8 l-chunks of 128 per tile
            tile_idx = 0
            for b in range(BL):
                for h in range(HT):
                    if b == 0:
                        # b0 in halves so tile 0 starts ASAP
                        T8b = stream.tile([128, C, LH], FP8, tag="stream")
                        nc.gpsimd.dma_start(
                            out=T8b,
                            in_=art8[0, :, ds(h * LH, LH)].rearrange(
                                "(c p) l -> p c l", p=128
                            ),
                        )
                        T8 = T8b
                    elif h == 0:
                        # whole-b fp8 article in one large-line transfer
                        T8b = stream.tile([128, C, LA], FP8, tag="stream")
                        nc.gpsimd.dma_start(
                            out=T8b,
                            in_=art8[b].rearrange("(c p) l -> p c l", p=128),
                        )
                        T8 = T8b[:, :, ds(0, LH)]
                    else:
                        T8 = T8b[:, :, ds(h * LH, LH)]
                    if tile_idx == 0:
                        # fp8 K-weight chunks co 1-3: one mid-size transfer
                        # (each gpsimd dma issue costs ~1us; seven per-chunk
                        # issues would starve the An/T8 streams behind them)
                        nc.gpsimd.dma_start(
                            out=wk8[:, :, ds(128, 384)],
                            in_=wKa[:, ds(128, 384)].rearrange(
                                "(c p) o -> p c o", p=128
                            ),
                        )
                    if tile_idx == 2:
                        nc.sync.dma_start(
                            out=wdk, in_=wKd.rearrange("(c p) o -> p c o", p=128)
                        )
                        for bb in range(BL):
                            nc.sync.dma_start(
                                out=OT[:, :, bb],
                                in_=optT[bb].rearrange("(c p) o l -> p c o l", p=128),
                            )
                    if tile_idx == 3:
                        nc.sync.dma_start(
                            out=wqv, in_=wQV.rearrange("(c p) o -> p c o", p=128)
                        )
                        nc.sync.dma_start(
                            out=On, in_=optN.rearrange("(lc p) hh -> p lc hh", p=128)
                        )
                        nc.sync.dma_start(out=fw, in_=fwd)
                        nc.sync.dma_start(out=fb, in_=fbd)
                        nc.sync.dma_start(out=mask32, in_=m32d)
                        nc.sync.dma_start(out=mask32T, in_=m32Td)
                    # natural-layout article for the PE weighted V-sum
                    An = nstream.tile([128, NLC, H], FP8, tag="nstream")
                    nc.gpsimd.dma_start(
                        out=An,
                        in_=art16[b, ds(h * LH, LH), :].rearrange(
                            "(lc p) hh -> p lc hh", p=128
                        ),
                    )
                    if tile_idx == 0:
                        # K-weight chunks co 4-7 ride behind tile 0's An
                        nc.gpsimd.dma_start(
                            out=wk8[:, :, ds(512, 512)],
                            in_=wKa[:, ds(512, 512)].rearrange(
                                "(c p) o -> p c o", p=128
                            ),
                        )
                    mt = mtp.tile([128, C, LH], FP8, tag="mt")
                    for co in range(C):
                        if tile_idx == 0:
                            bias_a_block(co)
                        kp = pacc.tile([128, LH], F32, tag="acc")
                        for lt in range(2):
                            for cp in range(CP):
                                nc.tensor.matmul(
                                    kp[:, ds(lt * 512, 512)],
                                    lhsT=wk8[:, ds(2 * cp, 2), ts(co, 128)],
                                    rhs=T8[:, ds(2 * cp, 2), ds(lt * 512, 512)],
                                    start=(cp == 0),
                                    stop=(cp == CP - 1),
                                    perf_mode=DR,
                                )
                        nc.scalar.activation(
                            mt[:, co], kp, AF.Tanh, bias=biasA[:, co, b : b + 1]
                        )
                    # score logits as PSUM columns; one closed group per lc
                    lgc = plgc.tile([128, NLC], F32, tag="lgc")
                    for lc in range(NLC):
                        for co in range(C):
                            nc.tensor.matmul(
                                lgc[:, lc : lc + 1],
                                lhsT=mt[:, co, ts(lc, 128)],
                                rhs=vwa[:, co : co + 1],
                                start=(co == 0),
                                stop=(co == C - 1),
                            )
                    sc = scp.tile([128, NLC], FP8, tag="sc")
                    nc.scalar.activation(sc, lgc, AF.Exp)
                    # weighted V-sum + softmax-sum on the PE; groups closed
                    # within this tile (columns h*C+co and 2C+h)
                    for co in range(C):
                        for lc in range(NLC):
                            nc.tensor.matmul(
                                pu[:, h * C + co : h * C + co + 1],
                                lhsT=An[:, lc, ts(co, 128)],
                                rhs=sc[:, lc : lc + 1],
                                start=(lc == 0),
                                stop=(lc == NLC - 1),
                            )
                    nc.tensor.matmul(
                        pu[0:1, ds(2 * C + h * NLC, NLC)],
                        lhsT=ones128,
                        rhs=sc,
                        start=True,
                        stop=True,
                    )
                    if h == HT - 1:
                        # finalize this batch item's u and softmax sum now so
                        # its options chain can run inside the loop
                        nc.vector.tensor_copy(uTun[:, :, b], pu[:, 0:C])
                        nc.vector.tensor_add(
                            uTun[:, :, b], uTun[:, :, b], pu[:, C : 2 * C]
                        )
                        nc.vector.tensor_reduce(
                            ssums[:, b : b + 1].unsqueeze(1),
                            pu[0:1, ds(2 * C, 2 * NLC)].unsqueeze(1),
                            axis=AX.X,
                            op=ALU.add,
                        )
                    # per-batch-item options work interleaved into the loop
                    for j in OPT_SCHED.get(tile_idx, []):
                        kpd_b(j)
                        opt_chain(j)
                    if tile_idx == 6:
                        kpd_b(3)
                    tile_idx += 1

            # ---------- options branch tail: last batch item + scores ------
            opt_chain(3)
            # option score logits as PSUM columns (5 ol-chunks of 128),
            # one closed group per chunk
            mdtf = mdt.rearrange("p c ol -> p (c ol)")
            lgdc = plgc.tile([128, NOPT], F32, tag="lgc")
            for lc in range(NOPT):
                for co in range(C):
                    nc.tensor.matmul(
                        lgdc[:, lc : lc + 1],
                        lhsT=mdtf[:, ds(co * OL + lc * 128, 128)],
                        rhs=vwd[:, co : co + 1],
                        start=(co == 0),
                        stop=(co == C - 1),
                    )
            sdc = scp.tile([128, NOPT], BF16, tag="sc")
            nc.scalar.activation(sdc, lgdc, AF.Exp)
            # per-(b,o) softmax sums: 32-row block sums via mask matmuls
            ssml = psml.tile([4, NOPT], F32, tag="sml")
            for lc in range(NOPT):
                nc.tensor.matmul(
                    ssml[:, lc : lc + 1],
                    lhsT=mask32,
                    rhs=sdc[:, lc : lc + 1],
                    start=True,
                    stop=True,
                )
            recs = one.tile([4, NOPT], F32, tag="recs")
            nc.vector.reciprocal(recs, ssml)
            recs16 = one.tile([4, NOPT], BF16, tag="recs16")
            nc.vector.tensor_copy(recs16, recs)
            # replicate 1/sum back to the 32-row blocks, normalize, and mask
            # into block-diagonal score columns
            prep = psml.tile([128, NOPT], F32, tag="sml")
            nc.tensor.matmul(prep, lhsT=mask32T, rhs=recs16, start=True, stop=True)
            sdnc = scp.tile([128, NOPT], BF16, tag="sc")
            nc.vector.tensor_mul(sdnc, sdc, prep)
            bd = one.tile([128, NOPT, 4], BF16, tag="bd")
            nc.vector.tensor_mul(
                bd,
                mask32.unsqueeze(1).broadcast_to((128, NOPT, 4)),
                sdnc.unsqueeze(-1).broadcast_to((128, NOPT, 4)),
            )
            # weighted V-sum on the PE: u_d[h, (b,o)] per (c, lc) block
            udps = pub.tile([128, C * BO], F32, tag="pu")
            for c in range(C):
                for lc in range(NOPT):
                    nc.tensor.matmul(
                        udps[:, ds(c * BO + 4 * lc, 4)],
                        lhsT=On[:, lc, ts(c, 128)],
                        rhs=bd[:, lc, :],
                        start=True,
                        stop=True,
                    )
            u16 = one.tile([128, C, BL, NOPT], BF16, tag="u16")
            nc.vector.tensor_copy(
                u16.rearrange("p c b o -> p c (b o)"),
                udps.rearrange("p (c bo) -> p c bo", c=C),
            )
            pout = psml.tile([BL, OUTP], F32, tag="sml")
            for c in range(C):
                for o in range(NOPT):
                    nc.tensor.matmul(
                        pout,
                        lhsT=u16[:, c, :, o],
                        rhs=fw[:, o, c, :],
                        start=(c == 0 and o == 0),
                        stop=(c == C - 1 and o == NOPT - 1),
                    )
            out_s = one.tile([BL, OUTP], F32, tag="out_s")
            nc.vector.tensor_add(out_s, pout, fb)
            nc.sync.dma_start(out=outd, in_=out_s[:, 0:OUT])

    nc.compile()
    return nc


@functools.lru_cache(maxsize=1)
def get_nc() -> bass.Bass:
    return build_nc()


def make_in_maps(inputs: dict) -> list[dict]:
    bf16 = ml_dtypes.bfloat16
    fp8 = ml_dtypes.float8_e4m3
    art = np.ascontiguousarray(np.asarray(inputs["article_contexts"], np.float32))
    qc = np.asarray(inputs["question_contexts"], np.float32)
    opt = np.ascontiguousarray(np.asarray(inputs["options_embeds"], np.float32))
    idx = np.asarray(inputs["answer_indices"]).astype(np.int64)

    def g(name):
        return np.asarray(inputs[name], np.float32)

    aQwT = np.ascontiguousarray(g("a_Qw").T).astype(fp8)
    aKwT = np.ascontiguousarray(g("a_Kw").T).astype(fp8)
    dKwT = np.ascontiguousarray(g("d_Kw").T).astype(bf16)
    # folded: aq -> options query projection
    Wqv = g("d_Qw") @ g("a_Vw")  # [H, H]
    qvwT = np.ascontiguousarray(Wqv.T).astype(fp8)
    bias_qv = g("d_Qw") @ g("a_Vb") + g("d_Qb") + g("d_Kb")  # [H]
    # folded: per-option final weights
    f_w = g("f_w")  # [OUT, 5H]
    dVwT = g("d_Vw").T
    Ff = np.stack(
        [dVwT @ f_w[:, o * H : (o + 1) * H].T for o in range(NOPT)], axis=0
    )  # [o, H_in, OUT]
    fb_new = g("f_b") + sum(
        f_w[:, o * H : (o + 1) * H] @ g("d_Vb") for o in range(NOPT)
    )  # [OUT]
    fwT = np.zeros((128, NOPT, C, OUTP), np.float32)
    fwT[:, :, :, :OUT] = Ff.reshape(NOPT, C, 128, OUT).transpose(2, 0, 1, 3)

    def colvec(v, dt):  # [H] -> [128, C] chunk-major
        return np.ascontiguousarray(
            np.asarray(v, np.float32).reshape(C, 128).T
        ).astype(dt)

    vwaT = colvec(g("a_vw").reshape(H), fp8)
    vwdT = colvec(g("d_vw").reshape(H), bf16)
    qkbT = colvec(g("a_Qb") + g("a_Kb"), np.float32)
    qvbT = colvec(bias_qv, np.float32)

    artT = np.ascontiguousarray(art.transpose(0, 2, 1)).astype(fp8)  # [B, H, LA]
    artN = art.astype(fp8)  # [B, LA, H] natural layout for the PE V-sum
    optT = np.ascontiguousarray(opt.transpose(0, 3, 1, 2)).astype(bf16)
    onehot = np.zeros((B, LQ), np.float32)
    onehot[np.arange(B), idx] = 1.0

    shared = dict(
        aQwT=aQwT, aKwT=aKwT, qvwT=qvwT, dKwT=dKwT,
        vwaT=vwaT, vwdT=vwdT, qkbT=qkbT, qvbT=qvbT,
        fwT=fwT.astype(bf16),
        fb=np.ascontiguousarray(
            np.tile(np.pad(fb_new.astype(np.float32), (0, 3)).reshape(1, 8), (BL, 1))
        ),
        m32=np.kron(np.eye(4, dtype=np.float32), np.ones((32, 1), np.float32)).astype(
            bf16
        ),
        m32T=np.kron(np.eye(4, dtype=np.float32), np.ones((1, 32), np.float32)).astype(
            bf16
        ),
    )
    in_maps = []
    for r in range(NCORES):
        s = slice(r * BL, (r + 1) * BL)
        m = dict(shared)
        m["art8"] = artT[s]
        m["art16"] = artN[s]
        m["optT"] = optT[s]
        m["optN"] = np.ascontiguousarray(opt[s].reshape(-1, H)).astype(bf16)
        m["qc"] = qc[s].astype(bf16)
        m["oh"] = np.ascontiguousarray(onehot[s].T).astype(bf16)
        in_maps.append(m)
    return in_maps


def run(inputs: dict, trace: bool = False, tmpdir=None):
    from concourse.bass_utils import run_bass_kernel_spmd

    nc = get_nc()
    in_maps = make_in_maps(inputs)
    res = run_bass_kernel_spmd(
        nc, in_maps, core_ids=list(range(NCORES)), trace=trace, tmpdir=tmpdir
    )
    out = np.concatenate([res.results[r]["out"] for r in range(NCORES)], axis=0)
    return out, res


def kernel(**inputs) -> np.ndarray:
    out, _ = run(inputs, trace=False)
    return out
